# revision 26
# baseline (speedup 1.0000x reference)
"""Multi-head causal attention (B=4, S=2048, D=1024, H=16) on 8 TRN2 NeuronCores.

Sharding: 2 cores per batch element, 8 heads (512 dims) per core.
Each core computes QKV projections for its head slice, causal attention,
and a partial output projection (its 512 rows of Wo). The host sums the
two partial outputs per batch element and adds (bo + bv @ Wo) once
(softmax rows sum to 1, so the bv term passes through Wo exactly).

Compute dtype: bf16 matmul inputs with fp32 PSUM accumulation.

Per-core dataflow (layouts chosen so nothing needs an on-device
transpose; X^T is built host-side):
  1. X^T [d x seq] chunks DMA'd straight in (bf16).
  2. Q^T, K^T [dim(512) x seq] = W^T @ X^T, V [seq x dim] stored bf16
     in `vaug` [128, kti, 8, 128]: per head, 64 all-ones columns then
     the 64 V dims.  The P^T @ V_aug matmul then emits the softmax row
     sums replicated on PSUM partitions 0-63 (a ready-made broadcast)
     and ctx^T on partitions 64-127.
  3. Per head-pair/query-block: S^T tiles [ks,qs] = K @ Q^T (diagonal
     k-tiles first; the two heads' matmuls land on disjoint PE row
     groups and run concurrently), exp on ACT -> bf16 P^T, causal
     triangle zeroed by gpsimd affine_select on the 4 diagonal tiles.
     ctx^T accumulated as V_aug^T @ P^T.  1/sums via the fast DVE
     reciprocal on the replicated-sums partitions (base partition 0,
     as the custom DVE op requires), one tensor_mul normalizes.
  4. out_partial = ctx^T.T @ Wo, copy to SBUF bf16, DMA out (biases
     are applied host-side in fp32).
Schedule: block N's scores are emitted before block N-1's ctx matmuls
(PE has score work while ACT runs block N-1's exps); QKV projection of
seq chunk qb+1 is spread across the 4 head-pair iterations of query
block qb; the output projection of block qb trails by three head-pair
iterations (in staggered halves, one per iteration) so its ctxT
inputs' normalization is off the critical path and the PE has filler
work in the ACT-heavy final iterations of each query block.
"""

import sys

import numpy as np


def _ensure_concourse():
    try:
        import concourse  # noqa: F401
    except ImportError:
        sys.path.insert(0, "/opt/trn_rl_repo")


_ensure_concourse()

B, S, D, H, HD = 4, 2048, 1024, 16, 64
DC = 512  # dims (= 8 heads) per core
N_CORES = 8

_nc_cache = None


def _build_bass():
    from contextlib import ExitStack

    import concourse.mybir as mybir
    import concourse.tile as tile
    from concourse import bacc

    f32 = mybir.dt.float32
    bf16 = mybir.dt.bfloat16
    Exp = mybir.ActivationFunctionType.Exp

    nc = bacc.Bacc(None, target_bir_lowering=False)

    xt_d = nc.dram_tensor("xt", [D, S], bf16, kind="ExternalInput")
    wq = nc.dram_tensor("wq", [D, DC], bf16, kind="ExternalInput")
    wk = nc.dram_tensor("wk", [D, DC], bf16, kind="ExternalInput")
    wv = nc.dram_tensor("wv", [D, DC], bf16, kind="ExternalInput")
    wo = nc.dram_tensor("wo", [DC, D], bf16, kind="ExternalInput")
    bq_d = nc.dram_tensor("bq", [128, 4], f32, kind="ExternalInput")
    bk_d = nc.dram_tensor("bk", [128, 4], f32, kind="ExternalInput")
    out = nc.dram_tensor("out", [S, D], bf16, kind="ExternalOutput")

    xt_r = xt_d[:, :].rearrange("(ko ki) s -> ki ko s", ki=128)  # [128,8,S]
    wq_r = wq[:, :].rearrange("(ko ki) n -> ki ko n", ki=128)  # [128,8,512]
    wk_r = wk[:, :].rearrange("(ko ki) n -> ki ko n", ki=128)
    wv_r = wv[:, :].rearrange("(ko ki) n -> ki ko n", ki=128)
    wo_r = wo[:, :].rearrange("(ko ki) n -> ki ko n", ki=128)  # [128,4,1024]
    our = out[:, :].rearrange("(so si) d -> si so d", si=128)

    with tile.TileContext(nc) as tc, ExitStack() as ctx:
        pers = ctx.enter_context(tc.tile_pool(name="pers", bufs=1))
        qt = pers.tile([128, 4, S], bf16, name="qt")  # Q^T: dim x seq
        ktt = pers.tile([128, 4, S], bf16, name="ktt")  # K^T: dim x seq
        # Per head: 64 ones-columns then the 64 V dims.  The ctx matmul's
        # PSUM output then carries the softmax row sums replicated on
        # partitions 0-63 (a ready-made broadcast for the normalization
        # multiply, base-partition 0 as the fast-reciprocal custom DVE op
        # requires) and ctx^T on partitions 64-127.
        vaug = pers.tile([128, 16, 8, 128], bf16, name="vaug")
        ctxT = pers.tile([128, 4, S], bf16, name="ctxT")
        wq_sb = pers.tile([128, 8, DC], bf16, name="wq_sb")
        wk_sb = pers.tile([128, 8, DC], bf16, name="wk_sb")
        wv_sb = pers.tile([128, 8, DC], bf16, name="wv_sb")
        wo_sb = pers.tile([128, 4, D], bf16, name="wo_sb")
        bq_sb = pers.tile([128, 4], f32, name="bq_sb")
        bk_sb = pers.tile([128, 4], f32, name="bk_sb")

        nc.gpsimd.memset(vaug[:, :, :, 0:64], 1.0)

        with (
            tc.tile_pool(name="xt", bufs=2) as xt_pool,
            tc.tile_pool(name="ptp", bufs=32) as pt_pool,
            tc.tile_pool(name="pps", bufs=2, space="PSUM") as pps,
            tc.tile_pool(name="sps", bufs=2, space="PSUM") as sps,
            tc.tile_pool(name="ups", bufs=2, space="PSUM") as ups,
            tc.tile_pool(name="recp", bufs=4) as rec_pool,
            tc.tile_pool(name="osb", bufs=4) as osb_pool,
        ):
            xt_chunks = {}

            def emit_xt_dma(sb):
                xt_chunk = xt_pool.tile([128, 8, 512], bf16, tag="xt")
                nc.sync.dma_start(xt_chunk, xt_r[:, :, sb * 512 : (sb + 1) * 512])
                xt_chunks[sb] = xt_chunk

            def emit_qkv_group(sb, g):
                """One of 12 projection groups for seq chunk sb.

                Groups 0-3: Q^T m-tile g; 4-7: K^T m-tile g-4;
                8-11: V rows tile g-8.
                """
                ssl = slice(sb * 512, (sb + 1) * 512)
                xt_chunk = xt_chunks[sb]
                if g < 8:
                    m = g % 4
                    w_sb, dst, b_sb = (
                        (wq_sb, qt, bq_sb) if g < 4 else (wk_sb, ktt, bk_sb)
                    )
                    p = pps.tile([128, 512], f32, tag="pj")
                    for kd in range(8):
                        nc.tensor.matmul(
                            p,
                            lhsT=w_sb[:, kd, m * 128 : (m + 1) * 128],
                            rhs=xt_chunk[:, kd, :],
                            start=(kd == 0),
                            stop=(kd == 7),
                        )
                    nc.vector.tensor_scalar_add(dst[:, m, ssl], p, b_sb[:, m : m + 1])
                else:
                    sv = g - 8
                    p = pps.tile([128, 512], f32, tag="pj")
                    for kd in range(8):
                        nc.tensor.matmul(
                            p,
                            lhsT=xt_chunk[:, kd, sv * 128 : (sv + 1) * 128],
                            rhs=wv_sb[:, kd, :],
                            start=(kd == 0),
                            stop=(kd == 7),
                        )
                    nc.vector.tensor_copy(
                        vaug[:, sb * 4 + sv, :, 64:128],
                        p[:, :].rearrange("p (h i) -> p h i", h=8),
                    )

            def emit_scores(hp, qb):
                """Score matmuls + exp for one (head-pair, query-block).

                Both heads of the pair go into one [128, 2, 512] PSUM tile
                (2 banks) so a single ACT exp covers them; the two matmuls
                target disjoint PE row groups (partitions 0-63 / 64-127)
                and can overlap.  Diagonal k-tiles run FIRST so their
                gpsimd triangle-mask is done well before the ctx matmuls
                need the tiles; the in-tile triangle is zeroed with
                affine_select (valid iff p <= local f) on the bf16 tile.
                """
                tiles = []
                ktis = list(range(4 * qb, 4 * qb + 4)) + list(range(4 * qb))
                for kti in ktis:
                    oi = kti - 4 * qb
                    qoff = max(oi, 0) * 128
                    w = 512 - qoff
                    ps = sps.tile([128, 2, 512], f32, tag="s")
                    for h2 in range(2):
                        base = h2 * 64
                        nc.tensor.matmul(
                            ps[:, h2, :w],
                            lhsT=ktt[
                                base : base + 64, hp, kti * 128 : (kti + 1) * 128
                            ],
                            rhs=qt[
                                base : base + 64, hp,
                                qb * 512 + qoff : (qb + 1) * 512,
                            ],
                            start=True,
                            stop=True,
                        )
                    p_t = pt_pool.tile([128, 2, 512], bf16, tag="p")
                    nc.scalar.activation(p_t[:, :, :w], ps[:, :, :w], Exp, scale=0.125)
                    if oi >= 0:
                        nc.gpsimd.affine_select(
                            out=p_t[:, :, :w],
                            in_=p_t[:, :, :w],
                            compare_op=mybir.AluOpType.is_ge,
                            fill=0.0,
                            base=0,
                            channel_multiplier=-1,
                            pattern=[[0, 2], [1, w]],
                        )
                    tiles.append((kti, qoff, w, p_t))
                return tiles

            def emit_ctx(hp, qb, tiles):
                """P^T @ V_aug accumulation + softmax normalization.

                u rows 0-63 hold the softmax row sums replicated (ones
                block of vaug); rows 64-127 are ctx^T for the head.  The
                fast DVE reciprocal on rows 0-63 directly yields the
                broadcast 1/sums; one tensor_mul normalizes.
                """
                nkt = len(tiles)
                qsl = slice(qb * 512, (qb + 1) * 512)
                for h2 in range(2):
                    h = 2 * hp + h2
                    u = ups.tile([128, 512], f32, tag="u")
                    for j, (kti, qoff, w, p_t) in enumerate(tiles):
                        nc.tensor.matmul(
                            u[:, qoff : qoff + w],
                            lhsT=vaug[:, kti, h, :],
                            rhs=p_t[:, h2, :w],
                            start=(j == 0),
                            stop=(j == nkt - 1),
                        )
                    rec = rec_pool.tile([64, 512], f32, tag="rec")
                    nc.vector.reciprocal_approx_fast(rec, u[0:64, :])
                    nc.vector.tensor_mul(
                        ctxT[h2 * 64 : h2 * 64 + 64, hp, qsl], u[64:128, :], rec
                    )

            def emit_outproj(qb, half=None):
                """Output projection for the seq tiles of query block qb."""
                tiles_ms = range(qb * 4, qb * 4 + 4)
                if half is not None:
                    tiles_ms = tiles_ms[half * 2 : half * 2 + 2]
                for ms in tiles_ms:
                    for nb in range(2):
                        po = pps.tile([128, 512], f32, tag="pj")
                        for kd in range(4):
                            nc.tensor.matmul(
                                po,
                                lhsT=ctxT[:, kd, ms * 128 : (ms + 1) * 128],
                                rhs=wo_sb[:, kd, nb * 512 : (nb + 1) * 512],
                                start=(kd == 0),
                                stop=(kd == 3),
                            )
                        ot = osb_pool.tile([128, 512], bf16, tag="ot")
                        if qb == 3:
                            # tail: ACT is idle after the last exp; keep the
                            # final evictions off the DVE queue
                            nc.scalar.copy(ot, po)
                        else:
                            nc.vector.tensor_copy(ot, po)
                        nc.sync.dma_start(
                            our[:, ms, nb * 512 : (nb + 1) * 512], ot
                        )

            # ---- prologue: DMAs + chunk-0 projections ----
            # xt chunk 0 and wq are split finely so the very first
            # projection matmuls can start as soon as their slices land.
            xt_chunk = xt_pool.tile([128, 8, 512], bf16, tag="xt")
            nc.sync.dma_start(xt_chunk[:, 0:1, :], xt_r[:, 0:1, 0:512])
            nc.scalar.dma_start(wq_sb[:, 0:2, :], wq_r[:, 0:2, :])
            nc.sync.dma_start(xt_chunk[:, 1:2, :], xt_r[:, 1:2, 0:512])
            nc.scalar.dma_start(wq_sb[:, 2:4, :], wq_r[:, 2:4, :])
            nc.sync.dma_start(xt_chunk[:, 2:4, :], xt_r[:, 2:4, 0:512])
            nc.sync.dma_start(xt_chunk[:, 4:8, :], xt_r[:, 4:8, 0:512])
            nc.scalar.dma_start(wq_sb[:, 4:8, :], wq_r[:, 4:8, :])
            xt_chunks[0] = xt_chunk
            nc.scalar.dma_start(bq_sb[:, :], bq_d[:, :])
            nc.scalar.dma_start(bk_sb[:, :], bk_d[:, :])
            nc.sync.dma_start(wk_sb[:, :, :], wk_r)
            nc.scalar.dma_start(wv_sb[:, :, :], wv_r)
            nc.scalar.dma_start(wo_sb[:, :, :], wo_r)
            for g in range(12):
                emit_qkv_group(0, g)

            # ---- pipelined attention ----
            # The output projection of block qb' trails by three head-pair
            # iterations, in halves: the second half of outproj(qb) then
            # lands in the final iteration of qb+1, where it is the PE's
            # filler work while ACT grinds through that block's exps.
            history = []
            pending = []  # deferred outproj halves, one per iteration
            for qb in range(4):
                if qb < 3:
                    emit_xt_dma(qb + 1)
                for hp in range(4):
                    tiles = emit_scores(hp, qb)
                    if qb < 3:
                        for g in range(3 * hp, 3 * hp + 3):
                            emit_qkv_group(qb + 1, g)
                    if history:
                        emit_ctx(*history[-1])
                    if len(history) >= 4 and history[-4][0] == 3:
                        pending += [(history[-4][1], 0), (history[-4][1], 1)]
                    if pending:
                        emit_outproj(*pending.pop(0))
                    history.append((hp, qb, tiles))
            # flush deferred outproj halves BEFORE the final ctx: they do
            # not depend on the last block's normalization, so they keep the
            # PE busy while ACT finishes the last block's exps.
            for qb_h in pending:
                emit_outproj(*qb_h)
            emit_ctx(*history[-1])
            emit_outproj(3)

    nc.finalize()
    return nc


def _get_nc():
    global _nc_cache
    if _nc_cache is None:
        _nc_cache = _build_bass()
    return _nc_cache


def make_in_maps(inputs, Wq, bq, Wk, bk, Wv, bv, Wo, bo):
    import ml_dtypes

    bf = ml_dtypes.bfloat16
    inputs = np.asarray(inputs, dtype=np.float32)
    Wq, Wk, Wv, Wo = (np.asarray(a, dtype=np.float32) for a in (Wq, Wk, Wv, Wo))
    bq, bk = (np.asarray(a, dtype=np.float32) for a in (bq, bk))
    in_maps = []
    for c in range(N_CORES):
        b = c // 2
        lo = (c % 2) * DC
        hi = lo + DC
        in_maps.append(
            {
                "xt": np.ascontiguousarray(inputs[b].T).astype(bf),
                "wq": np.ascontiguousarray(Wq[:, lo:hi]).astype(bf),
                "wk": np.ascontiguousarray(Wk[:, lo:hi]).astype(bf),
                "wv": np.ascontiguousarray(Wv[:, lo:hi]).astype(bf),
                "wo": np.ascontiguousarray(Wo[lo:hi, :]).astype(bf),
                "bq": np.ascontiguousarray(bq[lo:hi].reshape(4, 128).T),
                "bk": np.ascontiguousarray(bk[lo:hi].reshape(4, 128).T),
            }
        )
    return in_maps


def run(in_maps, bias_full, trace=False):
    from concourse.bass_utils import run_bass_kernel_spmd

    nc = _get_nc()
    res = run_bass_kernel_spmd(
        nc, in_maps, core_ids=list(range(N_CORES)), trace=trace
    )
    parts = [np.asarray(r["out"], dtype=np.float32) for r in res.results]
    full = np.stack([parts[2 * b] + parts[2 * b + 1] for b in range(B)])
    full += bias_full[None, None, :]
    return full, res


def _bias_full(Wo, bv, bo):
    # softmax rows sum to 1: ctx = attn @ (V + bv) = attn @ V + bv, and the
    # bv term passes through the output projection exactly.
    Wo = np.asarray(Wo, dtype=np.float32)
    bv = np.asarray(bv, dtype=np.float32)
    bo = np.asarray(bo, dtype=np.float32)
    return bo + bv @ Wo


def kernel(inputs, Wq, bq, Wk, bk, Wv, bv, Wo, bo):
    in_maps = make_in_maps(inputs, Wq, bq, Wk, bk, Wv, bv, Wo, bo)
    full, _ = run(in_maps, _bias_full(Wo, bv, bo), trace=False)
    return full
